# revision 6
# baseline (speedup 1.0000x reference)
"""Dimension-adaptive max pooling (8x6 bins) of (32, 64, 64, 512) fp32 images.

Data-parallel across 8 NeuronCores: each core pools 4 samples. Since the
spatial dims (64, 64) already exceed the bin counts (8, 6), the reference's
bilinear resize is the identity, so the op is pure irregular-bin max pooling:
  row (W) bins: uniform, 8 rows each
  col (H) bins: edges [0, 11, 21, 32, 43, 53, 64]  (round-half-even of i*64/6)
Output per sample: (8*6*512,) ordered [row_bin, col_bin, channel].

Per-core layout: rows r = k*8 + jhi*2 + jlo (k = row bin, jhi/jlo = position
within the bin). One DMA per 8-col chunk loads
  [128 partitions = jhi*32 + s*8 + k, (jlo, col, channel)]
(the shuffle comes free via the DRAM-side access pattern). VectorE
reduce_max(axis=XY) folds (jlo, cols-of-col-bin) per partition; col bins
spanning two chunks get a tensor_max merge. A 2-step partition tree
(128->64->32, always at 32-partition-aligned slices, killing jhi) leaves
[32 partitions = s*8 + k, 512] per col bin, which is DMA'd straight into the
output layout.
"""

import numpy as np

B, W, H, M = 32, 64, 64, 512
W_BINS, H_BINS = 8, 6
N_CORES = 8
PER_CORE = B // N_CORES  # 4
C_EDGES = [0, 11, 21, 32, 43, 53, 64]  # H-axis (col) bin edges
CHUNK = 8  # cols per DMA load

_PROG = None


def _build_program():
    import concourse.tile as tile
    from concourse import bacc, mybir

    f32 = mybir.dt.float32
    # Bacc (not raw Bass): its compile() runs generate_event_semaphores,
    # which splits multi-sem waits (TRN2 allows 1 wait per instruction).
    nc = bacc.Bacc()
    x = nc.declare_dram_parameter("x", [PER_CORE, W, H, M], f32, isOutput=False)
    z = nc.declare_dram_parameter(
        "z", [PER_CORE, W_BINS * H_BINS * M], f32, isOutput=True
    )
    # (sample, row_bin) -> partitions, (col_bin, channel) -> free
    zv = z.rearrange("s (k q) -> (s k) q", k=W_BINS)

    with tile.TileContext(nc) as tc:
        with (
            tc.tile_pool(name="chunks", bufs=3) as chunks,
            tc.tile_pool(name="binsp", bufs=1) as bins_pool,
            tc.tile_pool(name="tmps", bufs=2) as tmp_pool,
            tc.tile_pool(name="outs", bufs=1) as out_pool,
        ):
            bins = bins_pool.tile([128, H_BINS, M], f32)
            ot = out_pool.tile([32, H_BINS, M], f32)
            writes_seen = [0] * H_BINS
            for c0 in range(0, H, CHUNK):
                ht = chunks.tile([128, 2, CHUNK, M], f32)
                src = x[:, :, c0 : c0 + CHUNK, :].rearrange(
                    "s (k jhi jlo) c m -> jhi s k jlo c m", jhi=4, jlo=2
                )
                # one DMA per jlo slice keeps each AP within the 3-dim
                # balancing limit while still covering all 128 partitions
                for l in range(2):
                    nc.sync.dma_start(out=ht[:, l, :, :], in_=src[:, :, :, l, :, :])

                for b in range(H_BINS):
                    c1 = max(C_EDGES[b], c0)
                    c2 = min(C_EDGES[b + 1], c0 + CHUNK)
                    if c1 >= c2:
                        continue
                    # (jlo, cols) innermost -> reduce axis=XY keeps channels
                    rin = ht[:, :, c1 - c0 : c2 - c0, :].rearrange(
                        "p jlo c m -> p m jlo c"
                    )
                    writes_seen[b] += 1
                    if writes_seen[b] == 1:
                        nc.vector.reduce_max(
                            bins[:, b, :], rin, axis=mybir.AxisListType.XY
                        )
                    else:
                        tmp = tmp_pool.tile([128, M], f32)
                        nc.vector.reduce_max(tmp[:], rin, axis=mybir.AxisListType.XY)
                        nc.vector.tensor_max(bins[:, b, :], bins[:, b, :], tmp[:])
                    if C_EDGES[b + 1] <= c0 + CHUNK:
                        # col bin b complete: partition tree 128->64->32 kills
                        # jhi. tensor_tensor needs both SBUF inputs at the
                        # same base partition, so realign with ScalarE copies
                        # (single-input ops may shift by 32-partition steps).
                        t64 = tmp_pool.tile([64, M], f32, tag="t64")
                        nc.scalar.copy(t64[:], bins[64:128, b, :])
                        nc.vector.tensor_max(
                            bins[0:64, b, :], bins[0:64, b, :], t64[:]
                        )
                        t32 = tmp_pool.tile([32, M], f32, tag="t32")
                        nc.scalar.copy(t32[:], bins[32:64, b, :])
                        nc.vector.tensor_max(
                            ot[:, b, :], bins[0:32, b, :], t32[:]
                        )
                        nc.scalar.dma_start(
                            out=zv[:, b * M : (b + 1) * M], in_=ot[:, b, :]
                        )
    nc.compile()  # bacc lowering: reg alloc + multi-wait splitting
    return nc


def _get_program():
    global _PROG
    if _PROG is None:
        _PROG = _build_program()
    return _PROG


def run(xp, trace=False):
    """Run on 8 NeuronCores. Returns (z, BassKernelResults)."""
    from concourse.bass_utils import run_bass_kernel_spmd

    xp = np.ascontiguousarray(np.asarray(xp, dtype=np.float32))
    assert xp.shape == (B, W, H, M), xp.shape
    nc = _get_program()
    in_maps = [
        {"x": np.ascontiguousarray(xp[i * PER_CORE : (i + 1) * PER_CORE])}
        for i in range(N_CORES)
    ]
    res = run_bass_kernel_spmd(nc, in_maps, list(range(N_CORES)), trace=trace)
    z = np.concatenate([r["z"] for r in res.results], axis=0)
    return z, res


def kernel(xp) -> np.ndarray:
    z, _ = run(xp, trace=False)
    return z


# revision 7
# speedup vs baseline: 1.8782x; 1.8782x over previous
"""Dimension-adaptive max pooling (8x6 bins) of (32, 64, 64, 512) fp32 images.

Data-parallel across 8 NeuronCores: each core pools 4 samples. Since the
spatial dims (64, 64) already exceed the bin counts (8, 6), the reference's
bilinear resize is the identity, so the op is pure irregular-bin max pooling:
  row (W) bins: uniform, 8 rows each
  col (H) bins: edges [0, 11, 21, 32, 43, 53, 64]  (round-half-even of i*64/6)
Output per sample: (8*6*512,) ordered [row_bin, col_bin, channel].

Per-core layout: rows r = k*8 + jhi*2 + jlo (k = row bin, jhi/jlo = position
within the bin). One DMA per 8-col chunk loads
  [128 partitions = jhi*32 + s*8 + k, (jlo, col, channel)]
(the shuffle comes free via the DRAM-side access pattern). VectorE
reduce_max(axis=XY) folds (jlo, cols-of-col-bin) per partition; col bins
spanning two chunks get a tensor_max merge. A 2-step partition tree
(128->64->32, always at 32-partition-aligned slices, killing jhi) leaves
[32 partitions = s*8 + k, 512] per col bin, which is DMA'd straight into the
output layout.
"""

import numpy as np

B, W, H, M = 32, 64, 64, 512
W_BINS, H_BINS = 8, 6
N_CORES = 8
PER_CORE = B // N_CORES  # 4
C_EDGES = [0, 11, 21, 32, 43, 53, 64]  # H-axis (col) bin edges
CHUNK = 8  # cols per DMA load

_PROG = None


def _build_program():
    import concourse.tile as tile
    from concourse import bacc, mybir

    f32 = mybir.dt.float32
    # Bacc (not raw Bass): its compile() runs generate_event_semaphores,
    # which splits multi-sem waits (TRN2 allows 1 wait per instruction).
    nc = bacc.Bacc()
    x = nc.declare_dram_parameter("x", [PER_CORE, W, H, M], f32, isOutput=False)
    z = nc.declare_dram_parameter(
        "z", [PER_CORE, W_BINS * H_BINS * M], f32, isOutput=True
    )
    # (sample, row_bin) -> partitions, (col_bin, channel) -> free
    zv = z.rearrange("s (k q) -> (s k) q", k=W_BINS)

    with tile.TileContext(nc) as tc:
        with (
            tc.tile_pool(name="chunks", bufs=3) as chunks,
            tc.tile_pool(name="binsp", bufs=1) as bins_pool,
            tc.tile_pool(name="tmps", bufs=2) as tmp_pool,
            tc.tile_pool(name="outs", bufs=1) as out_pool,
        ):
            bins = bins_pool.tile([128, H_BINS, M], f32)
            ot = out_pool.tile([32, H_BINS, M], f32)
            writes_seen = [0] * H_BINS
            for c0 in range(0, H, CHUNK):
                ht = chunks.tile([128, 2, CHUNK, M], f32)
                src = x[:, :, c0 : c0 + CHUNK, :].rearrange(
                    "s (k jhi jlo) c m -> jhi s k jlo c m", jhi=4, jlo=2
                )
                # one DMA per jhi quarter: 3-dim APs, and the source's
                # outermost dim (s*k = 32) is what the HWDGE splits across
                # SDMA engines -- jhi-outer would use only 4 of 16 engines.
                for q in range(4):
                    nc.sync.dma_start(
                        out=ht[32 * q : 32 * (q + 1)], in_=src[q]
                    )

                for b in range(H_BINS):
                    c1 = max(C_EDGES[b], c0)
                    c2 = min(C_EDGES[b + 1], c0 + CHUNK)
                    if c1 >= c2:
                        continue
                    # (jlo, cols) innermost -> reduce axis=XY keeps channels
                    rin = ht[:, :, c1 - c0 : c2 - c0, :].rearrange(
                        "p jlo c m -> p m jlo c"
                    )
                    writes_seen[b] += 1
                    if writes_seen[b] == 1:
                        nc.vector.reduce_max(
                            bins[:, b, :], rin, axis=mybir.AxisListType.XY
                        )
                    else:
                        tmp = tmp_pool.tile([128, M], f32)
                        nc.vector.reduce_max(tmp[:], rin, axis=mybir.AxisListType.XY)
                        nc.vector.tensor_max(bins[:, b, :], bins[:, b, :], tmp[:])
                    if C_EDGES[b + 1] <= c0 + CHUNK:
                        # col bin b complete: partition tree 128->64->32 kills
                        # jhi. tensor_tensor needs both SBUF inputs at the
                        # same base partition, so realign with ScalarE copies
                        # (single-input ops may shift by 32-partition steps).
                        t64 = tmp_pool.tile([64, M], f32, tag="t64")
                        nc.scalar.copy(t64[:], bins[64:128, b, :])
                        nc.vector.tensor_max(
                            bins[0:64, b, :], bins[0:64, b, :], t64[:]
                        )
                        t32 = tmp_pool.tile([32, M], f32, tag="t32")
                        nc.scalar.copy(t32[:], bins[32:64, b, :])
                        nc.vector.tensor_max(
                            ot[:, b, :], bins[0:32, b, :], t32[:]
                        )
                        nc.scalar.dma_start(
                            out=zv[:, b * M : (b + 1) * M], in_=ot[:, b, :]
                        )
    nc.compile()  # bacc lowering: reg alloc + multi-wait splitting
    return nc


def _get_program():
    global _PROG
    if _PROG is None:
        _PROG = _build_program()
    return _PROG


def run(xp, trace=False):
    """Run on 8 NeuronCores. Returns (z, BassKernelResults)."""
    from concourse.bass_utils import run_bass_kernel_spmd

    xp = np.ascontiguousarray(np.asarray(xp, dtype=np.float32))
    assert xp.shape == (B, W, H, M), xp.shape
    nc = _get_program()
    in_maps = [
        {"x": np.ascontiguousarray(xp[i * PER_CORE : (i + 1) * PER_CORE])}
        for i in range(N_CORES)
    ]
    res = run_bass_kernel_spmd(nc, in_maps, list(range(N_CORES)), trace=trace)
    z = np.concatenate([r["z"] for r in res.results], axis=0)
    return z, res


def kernel(xp) -> np.ndarray:
    z, _ = run(xp, trace=False)
    return z
